# revision 4
# baseline (speedup 1.0000x reference)
"""Cross-attention (softmax over queries) on 8 Trainium2 NeuronCores.

Reference (per batch b):
    q = y @ Wq.T + bq            [N, H]
    k = x @ Wk.T + bk            [M, H]
    v = x @ Wv.T + bv            [M, D]
    dots = (q @ k.T) * H**-0.5   [N, M]
    attn = softmax(dots, axis=0) (over queries n, per key column m)
    out  = attn @ v              [N, D]

Sharding: data-parallel over batch B=8, one batch per core (SPMD).

Device algorithm (per core; matmuls f16/bf16 with fp32 PSUM accumulation):
  Host pre-transposes/casts y,x to yT,xT f16 and weights to f16 (Wq
  pre-scaled by H**-0.5), so no on-device transposes are needed.
  A. qT[h,n] / kT[h,m] projections straight from DMA'd yT/xT; q/k biases
     added by the ACT psum->sbuf copy (per-partition).
  C. per 128-row key chunk mc: V-projection chunk (bias via K=1 ones
     matmul), dotsT[m,n] into two [128,1024] PSUM halves, then
     attnT = exp(dots - 32) on ACT (bf16 out, f32 row-sum accumulated;
     no per-column max: the constant shift keeps exp args <= 0 and the
     softmax normalization cancels it; bf16 range absorbs the scale),
     fold 1/colsum into v via one ACT mul (psum -> bf16).
  D. out[n,d] = sum_m attnT[m,n] * v'[m,d]; dense 16-matmul PSUM chains.
"""

from contextlib import ExitStack

import numpy as np

import concourse.mybir as mybir
import concourse.tile as tile
from concourse import bacc
from concourse.bass_utils import run_bass_kernel_spmd

F32 = mybir.dt.float32
F16 = mybir.dt.float16
BF16 = mybir.dt.bfloat16
Exp = mybir.ActivationFunctionType.Exp
AX = mybir.AxisListType.X

B, N, M, C, H, D = 8, 2048, 2048, 1024, 512, 1024
P = 128
NT, MT, CCH, HC = N // P, M // P, C // P, H // P  # 16, 16, 8, 4
SCALE = (C // 2) ** -0.5
SHIFT = -32.0  # exp(dots + SHIFT): keeps exp args <= 0; cancels in softmax

_CACHE = {}


def _build_nc():
    nc = bacc.Bacc("TRN2", target_bir_lowering=False, debug=False)

    yt_d = nc.dram_tensor("yt", [C, N], F16, kind="ExternalInput").ap()
    xt_d = nc.dram_tensor("xt", [C, M], F16, kind="ExternalInput").ap()
    wqt_d = nc.dram_tensor("wqt", [C, H], F16, kind="ExternalInput").ap()
    wkt_d = nc.dram_tensor("wkt", [C, H], F16, kind="ExternalInput").ap()
    wvt_d = nc.dram_tensor("wvt", [C, D], F16, kind="ExternalInput").ap()
    bq_d = nc.dram_tensor("bq", [H], F32, kind="ExternalInput").ap()
    bk_d = nc.dram_tensor("bk", [H], F32, kind="ExternalInput").ap()
    bv_d = nc.dram_tensor("bv", [D], F16, kind="ExternalInput").ap()
    out_d = nc.dram_tensor("out", [N, D], F32, kind="ExternalOutput").ap()

    yt_r = yt_d.rearrange("(o p) n -> p o n", p=P)  # [128, 8, 2048]
    xt_r = xt_d.rearrange("(o p) m -> p o m", p=P)
    out_r = out_d.rearrange("(t p) d -> p t d", p=P)

    with tile.TileContext(nc) as tc:
        with (
            tc.tile_pool(name="persist", bufs=1) as pers,
            tc.tile_pool(name="stats", bufs=1) as stats,
            tc.tile_pool(name="xT_pool", bufs=1) as xTp,
        ):
            # ps_pp spans phases A+C; closed explicitly before phase D
            pp_stack = ExitStack()
            psPP = pp_stack.enter_context(
                tc.tile_pool(name="ps_pp", bufs=4, space="PSUM")
            )
            qT = pers.tile([P, HC, N], F16, tag="qT")  # [h%128, h//128, n] 2MB
            kT = pers.tile([P, HC, M], F16, tag="kT")  # 2MB
            v = pers.tile([P, MT, D], BF16, tag="v")  # [m%128, m//128, d] 4MB
            ones = pers.tile([1, P], F16, tag="ones")
            nc.vector.memset(ones[:], 1.0)
            shift = pers.tile([P, 1], F32, tag="shift")
            nc.vector.memset(shift[:], SHIFT)

            ssum = stats.tile([P, MT, 2], F32, tag="ssum")
            sums = stats.tile([P, MT], F32, tag="sums")
            rsum = stats.tile([P, MT], F32, tag="rsum")
            bq_sb = stats.tile([P, HC], F32, tag="bq")  # [h%128, h//128]
            bk_sb = stats.tile([P, HC], F32, tag="bk")
            bv_sb = stats.tile([1, D], F16, tag="bv")
            nc.sync.dma_start(bq_sb[:], bq_d.rearrange("(o p) -> p o", p=P))
            nc.sync.dma_start(bk_sb[:], bk_d.rearrange("(o p) -> p o", p=P))
            nc.sync.dma_start(bv_sb[:], bv_d[None, :])

            xT = xTp.tile([P, CCH, M], F16, tag="xT")  # alive through phase C
            # stream xT in 4 column blocks on the scalar HWDGE queue
            for j in range(4):
                nc.scalar.dma_start(
                    xT[:, :, j * 512 : (j + 1) * 512],
                    xt_r[:, :, j * 512 : (j + 1) * 512],
                )

            # ---------- Phase A: q/k projections ----------
            with (
                tc.tile_pool(name="yT_pool", bufs=1) as yTp,
                tc.tile_pool(name="w_pool", bufs=1) as wp,
            ):
                wq_sb = wp.tile([P, CCH, H], F16, tag="wq")  # [c%128, c//128, h]
                wk_sb = wp.tile([P, CCH, H], F16, tag="wk")
                nc.sync.dma_start(wq_sb[:], wqt_d.rearrange("(o p) h -> p o h", p=P))
                nc.scalar.dma_start(wk_sb[:], wkt_d.rearrange("(o p) h -> p o h", p=P))

                yT = yTp.tile([P, CCH, N], F16, tag="yT")
                for j in range(4):
                    nc.sync.dma_start(
                        yT[:, :, j * 512 : (j + 1) * 512],
                        yt_r[:, :, j * 512 : (j + 1) * 512],
                    )

                def project_j(dst, w_sb, b_sb, src_T, j):
                    # one 512-wide column block of a projection, all hc chunks
                    for hc in range(HC):
                        pp = psPP.tile([P, 512], F32, tag="pp")
                        for cc in range(CCH):
                            nc.tensor.matmul(
                                pp[:],
                                w_sb[:, cc, hc * P : (hc + 1) * P],
                                src_T[:, cc, j * 512 : (j + 1) * 512],
                                start=(cc == 0),
                                stop=(cc == CCH - 1),
                            )
                        # ACT copy: psum -> f16, + per-partition bias
                        nc.scalar.add(
                            dst[:, hc, j * 512 : (j + 1) * 512],
                            pp[:],
                            b_sb[:, hc : hc + 1],
                        )

                with nc.named_scope("A_yq"):
                    for j in range(4):
                        project_j(qT, wq_sb, bq_sb, yT, j)
                with nc.named_scope("A_xk"):
                    for j in range(4):
                        project_j(kT, wk_sb, bk_sb, xT, j)

            # ---------- Phase C: V-proj chunks interleaved with dots/softmax ----------
            with (
                tc.tile_pool(name="late", bufs=1) as late,
            ):
                psC_stack = ExitStack()
                psC = psC_stack.enter_context(
                    tc.tile_pool(name="ps_c", bufs=1, space="PSUM")
                )
                attnT = late.tile([P, MT, N], BF16, tag="attnT")  # 8MB
                wv_sb = late.tile([P, CCH, D], F16, tag="wv")  # 2MB
                nc.sync.dma_start(wv_sb[:], wvt_d.rearrange("(o p) d -> p o d", p=P))

                def chunk(mc):
                    # v[m, d] for m-chunk mc: lhsT = xT (c,m), rhs = wv (c,d)
                    pvs = []
                    for dh in range(2):
                        pv = psPP.tile([P, 512], F32, tag="pp")
                        for cc in range(CCH):
                            nc.tensor.matmul(
                                pv[:],
                                xT[:, cc, mc * P : (mc + 1) * P],
                                wv_sb[:, cc, dh * 512 : (dh + 1) * 512],
                                start=(cc == 0),
                                stop=False,
                            )
                        nc.tensor.matmul(
                            pv[:],
                            ones[:, :P],
                            bv_sb[:, dh * 512 : (dh + 1) * 512],
                            start=False,
                            stop=True,
                        )
                        pvs.append(pv)
                    # dotsT[m, n] in two 1024-wide PSUM halves; exp each half
                    # as soon as its matmuls finish (no column max needed)
                    for h in range(2):
                        pd = psC.tile([P, 1024], F32, tag=f"dots{h}")
                        for j2 in range(2):
                            j = h * 2 + j2
                            for hc in range(HC):
                                nc.tensor.matmul(
                                    pd[:, j2 * 512 : (j2 + 1) * 512],
                                    kT[:, hc, mc * P : (mc + 1) * P],
                                    qT[:, hc, j * 512 : (j + 1) * 512],
                                    start=(hc == 0),
                                    stop=(hc == HC - 1),
                                )
                        nc.scalar.activation(
                            out=attnT[:, mc, h * 1024 : (h + 1) * 1024],
                            in_=pd[:],
                            func=Exp,
                            bias=shift[:],
                            accum_out=ssum[:, mc, h : h + 1],
                        )
                    nc.vector.tensor_tensor(
                        sums[:, mc : mc + 1],
                        ssum[:, mc, 0:1],
                        ssum[:, mc, 1:2],
                        mybir.AluOpType.add,
                    )
                    nc.vector.reciprocal(rsum[:, mc : mc + 1], sums[:, mc : mc + 1])
                    # fold 1/colsum into v rows for this m-chunk (psum -> bf16)
                    for dh in range(2):
                        nc.scalar.mul(
                            v[:, mc, dh * 512 : (dh + 1) * 512],
                            pvs[dh][:],
                            rsum[:, mc : mc + 1],
                        )

                with nc.named_scope("C_loop"):
                    for mc in range(MT):
                        chunk(mc)
                psC_stack.close()
                pp_stack.close()

                # ---------- Phase D: out = attnT^T @ v' ----------
                with (
                    tc.tile_pool(name="ps_d", bufs=4, space="PSUM") as psD,
                    tc.tile_pool(name="so", bufs=4) as so,
                    nc.named_scope("D_out"),
                ):
                    for ntc in range(NT):
                        for dh in range(2):
                            po = psD.tile([P, 512], F32, tag="po")
                            for mc in range(MT):
                                nc.tensor.matmul(
                                    po[:],
                                    attnT[:, mc, ntc * P : (ntc + 1) * P],
                                    v[:, mc, dh * 512 : (dh + 1) * 512],
                                    start=(mc == 0),
                                    stop=(mc == MT - 1),
                                )
                            ot = so.tile([P, 512], F32, tag="ot")
                            nc.scalar.copy(ot[:], po[:])
                            nc.sync.dma_start(
                                out_r[:, ntc, dh * 512 : (dh + 1) * 512], ot[:]
                            )

    nc.finalize()
    return nc


def _get_nc():
    if "nc" not in _CACHE:
        _CACHE["nc"] = _build_nc()
    return _CACHE["nc"]


def _prep_in_maps(y, x, Wq, bq, Wk, bk, Wv, bv):
    y16 = np.asarray(y, dtype=np.float16)
    x16 = np.asarray(x, dtype=np.float16)
    yt = np.ascontiguousarray(y16.transpose(0, 2, 1))  # [B, C, N]
    xt = np.ascontiguousarray(x16.transpose(0, 2, 1))
    wqt = np.ascontiguousarray((np.asarray(Wq) * SCALE).T.astype(np.float16))
    wkt = np.ascontiguousarray(np.asarray(Wk).T.astype(np.float16))
    wvt = np.ascontiguousarray(np.asarray(Wv).T.astype(np.float16))
    bq32 = (np.asarray(bq) * SCALE).astype(np.float32)
    bk32 = np.asarray(bk, dtype=np.float32)
    bv16 = np.asarray(bv).astype(np.float16)
    return [
        {
            "yt": yt[b],
            "xt": xt[b],
            "wqt": wqt,
            "wkt": wkt,
            "wvt": wvt,
            "bq": bq32,
            "bk": bk32,
            "bv": bv16,
        }
        for b in range(B)
    ]


def run(inputs, trace=False, trace_cores=None):
    nc = _get_nc()
    in_maps = _prep_in_maps(**inputs)
    r = run_bass_kernel_spmd(
        nc, in_maps, list(range(B)), trace=trace, trace_cores=trace_cores
    )
    out = np.stack([r.results[b]["out"] for b in range(B)], axis=0)
    return out, r


def kernel(**inputs) -> np.ndarray:
    out, _ = run(inputs, trace=False)
    return out


# revision 10
# speedup vs baseline: 1.0422x; 1.0422x over previous
"""Cross-attention (softmax over queries) on 8 Trainium2 NeuronCores.

Reference (per batch b):
    q = y @ Wq.T + bq            [N, H]
    k = x @ Wk.T + bk            [M, H]
    v = x @ Wv.T + bv            [M, D]
    dots = (q @ k.T) * H**-0.5   [N, M]
    attn = softmax(dots, axis=0) (over queries n, per key column m)
    out  = attn @ v              [N, D]

Sharding: data-parallel over batch B=8, one batch per core (SPMD).

Device algorithm (per core; matmuls f16/bf16 with fp32 PSUM accumulation):
  Host pre-transposes/casts y,x to yT,xT f16 and weights to f16 (Wq
  pre-scaled by H**-0.5), so no on-device transposes are needed.
  A. qT[h,n] / kT[h,m] projections straight from DMA'd yT/xT; q/k biases
     added by the ACT psum->sbuf copy (per-partition).
  C. per 128-row key chunk mc: V-projection chunk (bias via K=1 ones
     matmul), dotsT[m,n] into two [128,1024] PSUM halves, then
     attnT = exp(dots - 32) on ACT (bf16 out, f32 row-sum accumulated;
     no per-column max: the constant shift keeps exp args <= 0 and the
     softmax normalization cancels it; bf16 range absorbs the scale),
     fold 1/colsum into v via one ACT mul (psum -> bf16).
  D. out[n,d] = sum_m attnT[m,n] * v'[m,d]; dense 16-matmul PSUM chains.
"""

from contextlib import ExitStack

import numpy as np

import concourse.mybir as mybir
import concourse.tile as tile
from concourse import bacc
from concourse.bass_utils import run_bass_kernel_spmd

F32 = mybir.dt.float32
F16 = mybir.dt.float16
BF16 = mybir.dt.bfloat16
Exp = mybir.ActivationFunctionType.Exp
AX = mybir.AxisListType.X

B, N, M, C, H, D = 8, 2048, 2048, 1024, 512, 1024
P = 128
NT, MT, CCH, HC = N // P, M // P, C // P, H // P  # 16, 16, 8, 4
SCALE = (C // 2) ** -0.5
SHIFT = -32.0  # exp(dots + SHIFT): keeps exp args <= 0; cancels in softmax

_CACHE = {}


def _build_nc():
    nc = bacc.Bacc("TRN2", target_bir_lowering=False, debug=False)

    # activations/weights host-packed so every DMA is contiguous per
    # partition (large descriptors): yt[p, j, o, n'] = y.T[o*128+p, j*512+n']
    yt_d = nc.dram_tensor("yt", [P, 4, CCH, 512], F16, kind="ExternalInput").ap()
    xt_d = nc.dram_tensor("xt", [P, 4, CCH, 512], F16, kind="ExternalInput").ap()
    wqt_d = nc.dram_tensor("wqt", [P, CCH, H], F16, kind="ExternalInput").ap()
    wkt_d = nc.dram_tensor("wkt", [P, CCH, H], F16, kind="ExternalInput").ap()
    wvt_d = nc.dram_tensor("wvt", [P, CCH, D], F16, kind="ExternalInput").ap()
    bq_d = nc.dram_tensor("bq", [H], F32, kind="ExternalInput").ap()
    bk_d = nc.dram_tensor("bk", [H], F32, kind="ExternalInput").ap()
    bv_d = nc.dram_tensor("bv", [D], F16, kind="ExternalInput").ap()
    out_d = nc.dram_tensor("out", [N, D], F32, kind="ExternalOutput").ap()

    out_r = out_d.rearrange("(t p) d -> p t d", p=P)

    with tile.TileContext(nc) as tc:
        with (
            tc.tile_pool(name="persist", bufs=1) as pers,
            tc.tile_pool(name="stats", bufs=1) as stats,
            tc.tile_pool(name="xT_pool", bufs=1) as xTp,
        ):
            # ps_pp spans phases A+C; closed explicitly before phase D
            pp_stack = ExitStack()
            psPP = pp_stack.enter_context(
                tc.tile_pool(name="ps_pp", bufs=4, space="PSUM")
            )
            qT = pers.tile([P, HC, N], F16, tag="qT")  # [h%128, h//128, n] 2MB
            kT = pers.tile([P, HC, M], F16, tag="kT")  # 2MB
            v = pers.tile([P, MT, D], BF16, tag="v")  # [m%128, m//128, d] 4MB
            ones = pers.tile([1, P], F16, tag="ones")
            nc.vector.memset(ones[:], 1.0)
            shift = pers.tile([P, 1], F32, tag="shift")
            nc.vector.memset(shift[:], SHIFT)

            ssum = stats.tile([P, MT, 2], F32, tag="ssum")
            sums = stats.tile([P, MT], F32, tag="sums")
            rsum = stats.tile([P, MT], F32, tag="rsum")
            bq_sb = stats.tile([P, HC], F32, tag="bq")  # [h%128, h//128]
            bk_sb = stats.tile([P, HC], F32, tag="bk")
            bv_sb = stats.tile([1, D], F16, tag="bv")
            nc.scalar.dma_start(bq_sb[:], bq_d.rearrange("(o p) -> p o", p=P))
            nc.scalar.dma_start(bk_sb[:], bk_d.rearrange("(o p) -> p o", p=P))
            nc.scalar.dma_start(bv_sb[:], bv_d[None, :])

            xT = xTp.tile([P, 4, CCH, 512], F16, tag="xT")  # alive through phase C

            # ---------- Phase A: q/k projections ----------
            with (
                tc.tile_pool(name="yT_pool", bufs=1) as yTp,
                tc.tile_pool(name="w_pool", bufs=1) as wp,
            ):
                wq_sb = wp.tile([P, CCH, H], F16, tag="wq")  # [c%128, c//128, h]
                wk_sb = wp.tile([P, CCH, H], F16, tag="wk")
                # sync queue: wq first (phase A start), then yT blocks, wv
                nc.sync.dma_start(wq_sb[:], wqt_d)
                yT = yTp.tile([P, 4, CCH, 512], F16, tag="yT")
                for j in range(4):
                    nc.sync.dma_start(yT[:, j], yt_d[:, j])
                # scalar queue: wk + biases, then xT blocks
                nc.scalar.dma_start(wk_sb[:], wkt_d)
                for j in range(4):
                    nc.scalar.dma_start(xT[:, j], xt_d[:, j])

                def project_j(dst, w_sb, b_sb, src_T, j):
                    # one 512-wide column block of a projection, all hc chunks
                    for hc in range(HC):
                        pp = psPP.tile([P, 512], F32, tag="pp")
                        for cc in range(CCH):
                            nc.tensor.matmul(
                                pp[:],
                                w_sb[:, cc, hc * P : (hc + 1) * P],
                                src_T[:, j, cc, :],
                                start=(cc == 0),
                                stop=(cc == CCH - 1),
                            )
                        # ACT copy: psum -> f16, + per-partition bias
                        nc.scalar.add(
                            dst[:, hc, j * 512 : (j + 1) * 512],
                            pp[:],
                            b_sb[:, hc : hc + 1],
                        )

                with nc.named_scope("A_yq"):
                    for j in range(4):
                        project_j(qT, wq_sb, bq_sb, yT, j)
                with nc.named_scope("A_xk"):
                    for j in range(4):
                        project_j(kT, wk_sb, bk_sb, xT, j)

            # ---------- Phase C: V-proj chunks interleaved with dots/softmax ----------
            with (
                tc.tile_pool(name="late", bufs=1) as late,
            ):
                psC_stack = ExitStack()
                psC = psC_stack.enter_context(
                    tc.tile_pool(name="ps_c", bufs=1, space="PSUM")
                )
                attnT = late.tile([P, MT, N], BF16, tag="attnT")  # 8MB
                wv_sb = late.tile([P, CCH, D], F16, tag="wv")  # 2MB
                nc.sync.dma_start(wv_sb[:], wvt_d)

                def chunk(mc):
                    # dotsT[m, n] in two 1024-wide PSUM halves; exp each half
                    # as soon as its matmuls finish (no column max needed)
                    for h in range(2):
                        pd = psC.tile([P, 1024], F32, tag=f"dots{h}")
                        for j2 in range(2):
                            j = h * 2 + j2
                            for hc in range(HC):
                                nc.tensor.matmul(
                                    pd[:, j2 * 512 : (j2 + 1) * 512],
                                    kT[:, hc, mc * P : (mc + 1) * P],
                                    qT[:, hc, j * 512 : (j + 1) * 512],
                                    start=(hc == 0),
                                    stop=(hc == HC - 1),
                                )
                        nc.scalar.activation(
                            out=attnT[:, mc, h * 1024 : (h + 1) * 1024],
                            in_=pd[:],
                            func=Exp,
                            bias=shift[:],
                            accum_out=ssum[:, mc, h : h + 1],
                        )
                    nc.vector.tensor_tensor(
                        sums[:, mc : mc + 1],
                        ssum[:, mc, 0:1],
                        ssum[:, mc, 1:2],
                        mybir.AluOpType.add,
                    )
                    nc.vector.reciprocal(rsum[:, mc : mc + 1], sums[:, mc : mc + 1])
                    # v[m, d] for m-chunk mc: lhsT = xT (c,m), rhs = wv (c,d);
                    # after dots so the exp/fold tail hides under the matmuls
                    for dh in range(2):
                        pv = psPP.tile([P, 512], F32, tag="pp")
                        for cc in range(CCH):
                            nc.tensor.matmul(
                                pv[:],
                                xT[:, mc // 4, cc, (mc % 4) * P : (mc % 4 + 1) * P],
                                wv_sb[:, cc, dh * 512 : (dh + 1) * 512],
                                start=(cc == 0),
                                stop=False,
                            )
                        nc.tensor.matmul(
                            pv[:],
                            ones[:, :P],
                            bv_sb[:, dh * 512 : (dh + 1) * 512],
                            start=False,
                            stop=True,
                        )
                        # fold 1/colsum into v rows for this m-chunk (psum -> bf16)
                        nc.scalar.mul(
                            v[:, mc, dh * 512 : (dh + 1) * 512],
                            pv[:],
                            rsum[:, mc : mc + 1],
                        )

                with nc.named_scope("C_loop"):
                    for mc in range(MT):
                        chunk(mc)
                psC_stack.close()
                pp_stack.close()

                # ---------- Phase D: out = attnT^T @ v' ----------
                with (
                    tc.tile_pool(name="ps_d", bufs=4, space="PSUM") as psD,
                    tc.tile_pool(name="so", bufs=4) as so,
                    nc.named_scope("D_out"),
                ):
                    for ntc in range(NT):
                        for dh in range(2):
                            po = psD.tile([P, 512], F32, tag="po")
                            for mc in range(MT):
                                nc.tensor.matmul(
                                    po[:],
                                    attnT[:, mc, ntc * P : (ntc + 1) * P],
                                    v[:, mc, dh * 512 : (dh + 1) * 512],
                                    start=(mc == 0),
                                    stop=(mc == MT - 1),
                                )
                            ot = so.tile([P, 512], F32, tag="ot")
                            nc.scalar.copy(ot[:], po[:])
                            nc.sync.dma_start(
                                out_r[:, ntc, dh * 512 : (dh + 1) * 512], ot[:]
                            )

    nc.finalize()
    return nc


def _get_nc():
    if "nc" not in _CACHE:
        _CACHE["nc"] = _build_nc()
    return _CACHE["nc"]


def _pack_act(a16):
    # [B, N, C] f16 -> [B, p, j, o, n'] with a.T[o*128+p, j*512+n']
    at = a16.transpose(0, 2, 1).reshape(B, CCH, P, 4, 512)
    return np.ascontiguousarray(at.transpose(0, 2, 3, 1, 4))


def _pack_w(w):
    # [C, out] f16 -> [p, o, out]
    o = w.shape[1]
    return np.ascontiguousarray(w.reshape(CCH, P, o).transpose(1, 0, 2))


def _prep_in_maps(y, x, Wq, bq, Wk, bk, Wv, bv):
    yt = _pack_act(np.asarray(y, dtype=np.float16))
    xt = _pack_act(np.asarray(x, dtype=np.float16))
    wqt = _pack_w((np.asarray(Wq) * SCALE).T.astype(np.float16))
    wkt = _pack_w(np.asarray(Wk).T.astype(np.float16))
    wvt = _pack_w(np.asarray(Wv).T.astype(np.float16))
    bq32 = (np.asarray(bq) * SCALE).astype(np.float32)
    bk32 = np.asarray(bk, dtype=np.float32)
    bv16 = np.asarray(bv).astype(np.float16)
    return [
        {
            "yt": yt[b],
            "xt": xt[b],
            "wqt": wqt,
            "wkt": wkt,
            "wvt": wvt,
            "bq": bq32,
            "bk": bk32,
            "bv": bv16,
        }
        for b in range(B)
    ]


def run(inputs, trace=False, trace_cores=None):
    nc = _get_nc()
    in_maps = _prep_in_maps(**inputs)
    r = run_bass_kernel_spmd(
        nc, in_maps, list(range(B)), trace=trace, trace_cores=trace_cores
    )
    out = np.stack([r.results[b]["out"] for b in range(B)], axis=0)
    return out, r


def kernel(**inputs) -> np.ndarray:
    out, _ = run(inputs, trace=False)
    return out
